# revision 57
# baseline (speedup 1.0000x reference)
"""Causal single-head attention, data-parallel across 8 TRN2 NeuronCores.

Problem: x [512, 128, 512] f32, Wq/Wk/Wv [64, 512] f32.
  Q = x @ Wq.T; K = x @ Wk.T; V = x @ Wv.T     (per batch, [T=128, H=64])
  out = softmax(causal(Q K^T / 8)) @ V          ([T, H])

Sharding: batch dim (512) split across 8 cores, 64 batches/core, no
collectives.  Host prep (layout only): x is cast to bf16 and laid out
as [tile, c-part, c-chunk, token] so each 512-token tile (4 batches)
feeds N=512 matmuls directly; weights pre-transposed to [c, 3H] bf16
(unscaled -- the 1/8 softmax scale is folded into the exp activation).

Per-core kernel: 16 work items of 4 batches (512 tokens) each.  bf16
compute, bf16 output (host casts back to f32; adds ~3e-4 rel err).

  - QK projection: one [128,512]-out matmul per C-chunk (M=128: Q rows
    0-63, K rows 64-127), N=tokens amortizes the per-matmul overhead.
  - V projection lands directly in NATURAL [t, b, h] layout: per
    batch per chunk, lhsT = the x chunk (stationary, full-128-column
    weight load -> compiler FWL) and rhs = the Wv chunk (streams
    N=64).  Stream cost is 16x64 = 1024 rows/tile vs 2048 for the V^T
    form, and the PE transposes + ACT re-copy of the old V^T path
    disappear entirely (measured win ~2us over the V^T form).
  - PE matmul operands must share one base partition (walrus: "Fmap
    and Weight must start at the same partition"), but DVE/ACT copies
    CAN cross partition offsets (verified on HW), so K is evacuated
    from qk_ps partitions 64-127 straight to a base-0 tile by ACT.
  - scores are computed TRANSPOSED: S^T[s,t] = sum_h KT[h,s] QT[h,t]
    via lhsT=K^T, rhs=Q^T (both h-partitioned from the projection), so
    exp(S^T) IS P^T and feeds the PV matmul with no P transpose.
  - ACT exp applies the 1/8 scale; GPSIMD affine_select applies the
    causal mask (keep t >= s in [s, b, t] layout).
  - V gets a ones column appended so the PV matmul also produces the
    softmax row-sums; DVE reciprocal + broadcast multiply normalizes.
  - PE emission is INTERLEAVED across pipeline stages: QK-proj chunks
    of item i alternate with PV matmuls of item i-2, and V-proj
    matmuls of item i alternate with score matmuls of item i-1.  The
    PE weight port (LDWEIGHTS ~= cols/1.2 ns) runs in parallel with
    the rhs stream port with a 1-deep pull-ahead, so each small
    matmul's weight load hides under the preceding big matmul's
    stream time instead of serializing (HW-only win; the CoreSim cost
    model prices LDWEIGHTS at 0).
  - 3-stage software pipeline (proj i | mid i-1 | back i-2) keeps the
    PE queue from head-blocking on ACT/DVE work.
  - DVE queue order per iteration: q evac, vt evac, recip+mul of item
    i-2 (frees o_ps a full iteration before the next PV needs it --
    PSUM is bank-granular, 8 banks, all 8 in use, so `ops` cannot
    double-buffer).
  - Drain: the LAST item's exp/mask/normalize/DMA run as half-batch
    pairs, so the latency-bound final chain works on half-size data.
    (Splitting head/tail tiles into separate half work items, DVE or
    GPSIMD masking, a -32768 bias pre-accumulated in scores PSUM
    (GPSIMD cannot write PSUM; DVE/ACT writes overload them), extra
    DMA prefetch depth, half-tile x DMAs, and x loads on the SP ring
    were all tried and all regressed in TimelineSim -- see the session
    notes before re-trying.)
"""

import contextlib

import numpy as np
import ml_dtypes

import concourse.mybir as mybir
import concourse.tile as tile
from concourse import bacc
from concourse.bass_utils import run_bass_kernel_spmd

B, T, C, H = 512, 128, 512, 64
NCORES = 8
BPC = B // NCORES          # 64 batches per core
NBT = 4                    # batches per full token tile
NT = BPC // NBT            # 16 x-tiles
NTOK = NBT * T             # 512 tokens per full tile
KCH = C // 128             # 4 contraction chunks

BF16 = mybir.dt.bfloat16
F32 = mybir.dt.float32

# Work items: (x_tile, tok_off, nbt).  One item per x-tile; splitting
# head/tail tiles into half items was tried and lost 1.7-1.9us in sim
# (the drain chain is latency-bound and item splits add boundaries).
WORK = [(i, 0, NBT) for i in range(NT)]
NW = len(WORK)

_cache = {}


def _build(reps=1):
    nc = bacc.Bacc(
        "TRN2", target_bir_lowering=False, debug=False, enable_asserts=False
    )
    x_d = nc.dram_tensor(
        "x", [NT, 128, KCH, NTOK], BF16, kind="ExternalInput"
    ).ap()
    w_d = nc.dram_tensor("w", [128, KCH, 3 * H], BF16, kind="ExternalInput").ap()
    out_d = nc.dram_tensor("out", [NW, T, NBT, H], BF16, kind="ExternalOutput").ap()

    with tile.TileContext(nc) as tc:
        with (
            tc.tile_pool(name="const", bufs=1) as cpool,
            tc.tile_pool(name="xt", bufs=5) as xtpool,
            tc.tile_pool(name="qsb", bufs=2) as qpool,
            tc.tile_pool(name="ksb", bufs=2) as kpool,
            tc.tile_pool(name="vsb", bufs=3) as vpool,
            tc.tile_pool(name="psb", bufs=3) as ppool,
            tc.tile_pool(name="rsb", bufs=2) as rpool,
            tc.tile_pool(name="osb", bufs=2) as opool,
            tc.tile_pool(name="qkps", bufs=2, space="PSUM") as qkps,
            tc.tile_pool(name="vps", bufs=2, space="PSUM") as vps,
            tc.tile_pool(name="sps", bufs=2, space="PSUM") as sps,
            tc.tile_pool(name="ops", bufs=2, space="PSUM") as ops,
        ):
            w_sb = cpool.tile([128, KCH, 3 * H], BF16)
            nc.sync.dma_start(out=w_sb, in_=w_d)
            st = {}
            loaded = set()

            def dma_in(t, split=False):
                if t in loaded or t >= NT:
                    return
                loaded.add(t)
                xt = xtpool.tile([128, KCH, NTOK], BF16, tag="xt")
                if split:
                    # per-chunk DMAs, spread across HWDGE queues, so the
                    # first proj matmul only waits for 1/4 of the tile and
                    # descriptor generation overlaps (cold-start cut)
                    queues = [nc.scalar, nc.sync, nc.scalar, nc.sync]
                    for j in range(KCH):
                        queues[j].dma_start(out=xt[:, j, :], in_=x_d[t][:, j, :])
                else:
                    ring = nc.scalar if t % 2 == 0 else nc.sync
                    ring.dma_start(out=xt, in_=x_d[t])
                st[("x", t)] = xt

            def stage_proj_pe(w):
                """Emit QK+V proj matmuls for item w as generators so the
                caller can zipper them with older items' PE work."""
                xt_t, off, nbt = WORK[w]
                ntok = nbt * T
                xt = st[("x", xt_t)]
                s = st.setdefault(w, {})

                def qk_mms():
                    qk_ps = qkps.tile([128, NTOK], F32, tag="qk")
                    s["qk_ps"] = qk_ps
                    for j in range(KCH):
                        nc.tensor.matmul(
                            qk_ps[:, 0:ntok],
                            w_sb[:, j, 0:128],
                            xt[:, j, off : off + ntok],
                            start=(j == 0),
                            stop=(j == KCH - 1),
                            skip_group_check=True,
                        )
                        yield

                def v_mms():
                    # V in NATURAL [t, b, h] layout
                    v_ps = vps.tile([T, NBT, H], F32, tag="v")
                    s["v_ps"] = v_ps
                    for b in range(nbt):
                        for j in range(KCH):
                            nc.tensor.matmul(
                                v_ps[:, b, :],
                                xt[:, j, off + T * b : off + T * b + T],
                                w_sb[:, j, 128:192],
                                start=(j == 0),
                                stop=(j == KCH - 1),
                                skip_group_check=True,
                            )
                            yield

                return qk_mms(), v_mms()

            def stage_evac_qk(w):
                """DVE/ACT evacuation of item w's QK projection PSUM, and
                the Pool pre-write of the causal bias into the scores
                PSUM (runs during the proj phase, off the critical
                chain)."""
                _, _, nbt = WORK[w]
                ntok = nbt * T
                s = st[w]
                s_ps = sps.tile([T, NBT, T], F32, tag="s", name="s_ps")
                s["s_ps"] = s_ps
                q_sb = qpool.tile([64, NTOK], BF16, tag="q")
                nc.vector.tensor_copy(q_sb[:, 0:ntok], s["qk_ps"][0:64, 0:ntok])
                k_sb = kpool.tile([64, NTOK], BF16, tag="k")
                nc.scalar.copy(out=k_sb[:, 0:ntok], in_=s["qk_ps"][64:128, 0:ntok])
                s["q"], s["k"] = q_sb, k_sb

            def stage_evac_vt(w):
                _, _, nbt = WORK[w]
                s = st[w]
                v_sb = vpool.tile([T, NBT, H + 1], BF16, tag="v", name="v_sb")
                nc.vector.tensor_copy(
                    v_sb[:, 0:nbt, 0:H], s["v_ps"][:, 0:nbt, :]
                )
                nc.gpsimd.memset(v_sb[:, 0:nbt, H : H + 1], 1.0)
                s["v"] = v_sb

            def stage_mid_pe(w):
                """Transposes + score matmuls for item w (generator)."""
                _, _, nbt = WORK[w]
                s = st[w]

                def mms():
                    # transposed scores: S^T[s,t] per batch, K=64 contraction
                    s_ps = s["s_ps"]
                    for b in range(nbt):
                        nc.tensor.matmul(
                            s_ps[:, b, :],
                            s["k"][:, T * b : T * b + T],
                            s["q"][:, T * b : T * b + T],
                            start=True,
                            stop=True,
                            skip_group_check=True,
                        )
                        yield

                return mms()

            def stage_mid_post(w):
                """exp+mask (ACT/GPSIMD) and V-natural copy for item w.

                For the LAST item the exp is split into half-batches and
                the causal mask runs as a DVE multiply with the constant
                mask tile: the drain chain is latency-bound, and this
                shortens exp->mask->PV for the first half while keeping
                the (busy) Pool engine out of the critical path."""
                _, _, nbt = WORK[w]
                s = st[w]
                p_sb = ppool.tile([T, NBT, T], BF16, tag="p")
                s["p"] = p_sb
                if w < NW - 1:
                    halves = [(0, nbt)]
                else:
                    halves = [(0, nbt // 2), (nbt // 2, nbt)]
                for b0, b1 in halves:
                    nc.scalar.activation(
                        out=p_sb[:, b0:b1, :],
                        in_=s["s_ps"][:, b0:b1, :],
                        func=mybir.ActivationFunctionType.Exp,
                        scale=0.125,
                    )
                    # causal: keep where t - s >= 0 (layout [s, b, t])
                    nc.gpsimd.affine_select(
                        out=p_sb[:, b0:b1, :],
                        in_=p_sb[:, b0:b1, :],
                        pattern=[[0, b1 - b0], [1, T]],
                        compare_op=mybir.AluOpType.is_ge,
                        fill=0.0,
                        base=0,
                        channel_multiplier=-1,
                    )


            def stage_back_pe(w):
                """PV matmuls for item w (generator)."""
                _, _, nbt = WORK[w]
                s = st[w]

                def mms():
                    o_ps = ops.tile([T, NBT, H + 1], F32, tag="o")
                    s["o_ps"] = o_ps
                    for b in range(nbt):
                        nc.tensor.matmul(
                            o_ps[:, b, :],
                            s["p"][:, b, :],
                            s["v"][:, b, :],
                            start=True,
                            stop=True,
                            skip_group_check=True,
                        )
                        yield

                return mms()

            def stage_back_post(w):
                _, _, nbt = WORK[w]
                s = st[w]
                o_ps = s["o_ps"]
                r_sb = rpool.tile([T, NBT, 1], F32, tag="r")
                o_sb = opool.tile([T, NBT, H], BF16, tag="osb")
                if w < NW - 1:
                    halves = [(0, nbt)]
                else:
                    halves = [(0, nbt // 2), (nbt // 2, nbt)]
                for b0, b1 in halves:
                    nc.vector.reciprocal(
                        out=r_sb[:, b0:b1], in_=o_ps[:, b0:b1, H : H + 1]
                    )
                    nc.vector.tensor_mul(
                        o_sb[:, b0:b1, :],
                        o_ps[:, b0:b1, 0:H],
                        r_sb[:, b0:b1].to_broadcast([T, b1 - b0, H]),
                    )
                    nc.sync.dma_start(
                        out=out_d[w][:, b0:b1, :], in_=o_sb[:, b0:b1, :]
                    )
                del st[w]

            def chain(gen):
                if gen is not None:
                    for _ in gen:
                        pass

            def zipper(a, b):
                """Alternate PE emission between two generators."""
                gens = [g for g in (a, b) if g is not None]
                while gens:
                    alive = []
                    for g in gens:
                        try:
                            next(g)
                            alive.append(g)
                        except StopIteration:
                            pass
                    gens = alive

            loop = (
                tc.For_i(0, reps, 1, hint_engines=tuple(nc.engines))
                if reps > 1
                else contextlib.nullcontext()
            )
            with loop:
                for i in range(NW + 3):
                    if i == 0:
                        dma_in(0, split=True)
                        dma_in(1)
                    if i < NW:
                        # prefetch x-tiles two and three work items ahead
                        dma_in(WORK[min(i + 2, NW - 1)][0])
                        dma_in(WORK[min(i + 3, NW - 1)][0])
                    # PE order: PV (old, dep-safe) first, QK chunks
                    # back-to-back (their LDWEIGHTS hide under each
                    # other's streams), then V proj zippered with the
                    # score matmuls + transposes.
                    # DVE queue order: q evac first (scores need it
                    # mid-iteration), then the back-stage normalize
                    # (frees o_ps for the NEXT iteration's PV), then
                    # the vt evac (feeds the late transposes).
                    if 1 <= i <= NW:
                        stage_evac_qk(i - 1)
                        stage_evac_vt(i - 1)
                    qk = v = sc = None
                    if i < NW:
                        qk, v = stage_proj_pe(i)
                    if 1 <= i <= NW:
                        sc = stage_mid_pe(i - 1)
                    chain(qk)
                    # PV at depth i-3 (not i-2): the S->exp->mask chain
                    # (~2.5us) exceeds the ~1.6us tile period, so at
                    # depth 2 every PV stalls on the mask; depth 3 gives
                    # the chain a full extra iteration of slack
                    pv = stage_back_pe(i - 3) if i >= 3 else None
                    chain(pv)
                    if i >= 3:
                        stage_back_post(i - 3)
                    zipper(v, sc)
                    if 1 <= i <= NW:
                        stage_mid_post(i - 1)

    nc.compile()
    return nc


def _prep_inputs(x, Wq, Wk, Wv):
    w = np.concatenate(
        [np.asarray(Wq).T, np.asarray(Wk).T, np.asarray(Wv).T], axis=1
    )  # [C, 3H]
    w = np.ascontiguousarray(
        w.reshape(KCH, 128, 3 * H).transpose(1, 0, 2)
    ).astype(ml_dtypes.bfloat16)  # [128, KCH, 3H]
    # x [B, T, C] -> per-core [NT, 128(c-part), KCH, NTOK], token = b*T + t
    xt = np.asarray(x, dtype=np.float32).reshape(NCORES, NT, NBT, T, KCH, 128)
    xt = np.ascontiguousarray(xt.transpose(0, 1, 5, 4, 2, 3)).astype(
        ml_dtypes.bfloat16
    )
    xt = xt.reshape(NCORES, NT, 128, KCH, NTOK)
    return [{"x": xt[i], "w": w} for i in range(NCORES)]


def _run(in_maps, **kw):
    if "nc" not in _cache:
        _cache["nc"] = _build()
    return run_bass_kernel_spmd(
        _cache["nc"], in_maps, core_ids=list(range(NCORES)), **kw
    )


def kernel(x, Wq, Wk, Wv):
    res = _run(_prep_inputs(x, Wq, Wk, Wv))
    outs = []
    for r in res.results:
        o = r["out"].astype(np.float32)  # [NW, T, NBT, H]
        per_batch = []
        for w, (_, _, nbt) in enumerate(WORK):
            per_batch.append(o[w, :, 0:nbt, :].transpose(1, 0, 2))  # [nbt, T, H]
        outs.append(np.concatenate(per_batch, axis=0))  # [BPC, T, H]
    return np.ascontiguousarray(np.concatenate(outs, axis=0))


# revision 58
# speedup vs baseline: 1.0625x; 1.0625x over previous
"""Causal single-head attention, data-parallel across 8 TRN2 NeuronCores.

Problem: x [512, 128, 512] f32, Wq/Wk/Wv [64, 512] f32.
  Q = x @ Wq.T; K = x @ Wk.T; V = x @ Wv.T     (per batch, [T=128, H=64])
  out = softmax(causal(Q K^T / 8)) @ V          ([T, H])

Sharding: batch dim (512) split across 8 cores, 64 batches/core, no
collectives.  Host prep (layout only): x is cast to bf16 and laid out
as [tile, c-part, c-chunk, token] so each 512-token tile (4 batches)
feeds N=512 matmuls directly; weights pre-transposed to [c, 3H] bf16
(unscaled -- the 1/8 softmax scale is folded into the exp activation).

Per-core kernel: 16 work items of 4 batches (512 tokens) each.  bf16
compute, bf16 output (host casts back to f32; adds ~3e-4 rel err).

  - QK projection: one [128,512]-out matmul per C-chunk (M=128: Q rows
    0-63, K rows 64-127), N=tokens amortizes the per-matmul overhead.
  - V projection lands directly in NATURAL [t, b, h] layout: per
    batch per chunk, lhsT = the x chunk (stationary, full-128-column
    weight load -> compiler FWL) and rhs = the Wv chunk (streams
    N=64).  Stream cost is 16x64 = 1024 rows/tile vs 2048 for the V^T
    form, and the PE transposes + ACT re-copy of the old V^T path
    disappear entirely (measured win ~2us over the V^T form).
  - PE matmul operands must share one base partition (walrus: "Fmap
    and Weight must start at the same partition"), but DVE/ACT copies
    CAN cross partition offsets (verified on HW), so K is evacuated
    from qk_ps partitions 64-127 straight to a base-0 tile by ACT.
  - scores are computed TRANSPOSED: S^T[s,t] = sum_h KT[h,s] QT[h,t]
    via lhsT=K^T, rhs=Q^T (both h-partitioned from the projection), so
    exp(S^T) IS P^T and feeds the PV matmul with no P transpose.
  - ACT exp applies the 1/8 scale; GPSIMD affine_select applies the
    causal mask (keep t >= s in [s, b, t] layout).
  - V gets a ones column appended so the PV matmul also produces the
    softmax row-sums; DVE reciprocal + broadcast multiply normalizes.
  - PE emission is INTERLEAVED across pipeline stages: QK-proj chunks
    of item i alternate with PV matmuls of item i-2, and V-proj
    matmuls of item i alternate with score matmuls of item i-1.  The
    PE weight port (LDWEIGHTS ~= cols/1.2 ns) runs in parallel with
    the rhs stream port with a 1-deep pull-ahead, so each small
    matmul's weight load hides under the preceding big matmul's
    stream time instead of serializing (HW-only win; the CoreSim cost
    model prices LDWEIGHTS at 0).
  - 3-stage software pipeline (proj i | mid i-1 | back i-2) keeps the
    PE queue from head-blocking on ACT/DVE work.
  - DVE queue order per iteration: q evac, vt evac, recip+mul of item
    i-2 (frees o_ps a full iteration before the next PV needs it --
    PSUM is bank-granular, 8 banks, all 8 in use, so `ops` cannot
    double-buffer).
  - Drain: the LAST item's exp/mask/normalize/DMA run as half-batch
    pairs, so the latency-bound final chain works on half-size data.
    (Splitting head/tail tiles into separate half work items, DVE or
    GPSIMD masking, a -32768 bias pre-accumulated in scores PSUM
    (GPSIMD cannot write PSUM; DVE/ACT writes overload them), extra
    DMA prefetch depth, half-tile x DMAs, and x loads on the SP ring
    were all tried and all regressed in TimelineSim -- see the session
    notes before re-trying.)
"""

import contextlib

import numpy as np
import ml_dtypes

import concourse.mybir as mybir
import concourse.tile as tile
from concourse import bacc
from concourse.bass_utils import run_bass_kernel_spmd

B, T, C, H = 512, 128, 512, 64
NCORES = 8
BPC = B // NCORES          # 64 batches per core
NBT = 4                    # batches per full token tile
NT = BPC // NBT            # 16 x-tiles
NTOK = NBT * T             # 512 tokens per full tile
KCH = C // 128             # 4 contraction chunks

BF16 = mybir.dt.bfloat16
F32 = mybir.dt.float32

# Work items: (x_tile, tok_off, nbt).  One item per x-tile; splitting
# head/tail tiles into half items was tried and lost 1.7-1.9us in sim
# (the drain chain is latency-bound and item splits add boundaries).
WORK = [(i, 0, NBT) for i in range(NT)]
NW = len(WORK)

_cache = {}


def _build(reps=1):
    nc = bacc.Bacc(
        "TRN2", target_bir_lowering=False, debug=False, enable_asserts=False
    )
    x_d = nc.dram_tensor(
        "x", [NT, 128, KCH, NTOK], BF16, kind="ExternalInput"
    ).ap()
    w_d = nc.dram_tensor("w", [128, KCH, 3 * H], BF16, kind="ExternalInput").ap()
    out_d = nc.dram_tensor("out", [NW, T, NBT, H], BF16, kind="ExternalOutput").ap()

    with tile.TileContext(nc) as tc:
        with (
            tc.tile_pool(name="const", bufs=1) as cpool,
            tc.tile_pool(name="xt", bufs=5) as xtpool,
            tc.tile_pool(name="qsb", bufs=2) as qpool,
            tc.tile_pool(name="ksb", bufs=2) as kpool,
            tc.tile_pool(name="vsb", bufs=2) as vpool,
            tc.tile_pool(name="psb", bufs=2) as ppool,
            tc.tile_pool(name="rsb", bufs=2) as rpool,
            tc.tile_pool(name="osb", bufs=2) as opool,
            tc.tile_pool(name="qkps", bufs=2, space="PSUM") as qkps,
            tc.tile_pool(name="vps", bufs=2, space="PSUM") as vps,
            tc.tile_pool(name="sps", bufs=2, space="PSUM") as sps,
            tc.tile_pool(name="ops", bufs=2, space="PSUM") as ops,
        ):
            w_sb = cpool.tile([128, KCH, 3 * H], BF16)
            nc.sync.dma_start(out=w_sb, in_=w_d)
            st = {}
            loaded = set()

            def dma_in(t, split=False):
                if t in loaded or t >= NT:
                    return
                loaded.add(t)
                xt = xtpool.tile([128, KCH, NTOK], BF16, tag="xt")
                if split:
                    # per-chunk DMAs, spread across HWDGE queues, so the
                    # first proj matmul only waits for 1/4 of the tile and
                    # descriptor generation overlaps (cold-start cut)
                    queues = [nc.scalar, nc.sync, nc.scalar, nc.sync]
                    for j in range(KCH):
                        queues[j].dma_start(out=xt[:, j, :], in_=x_d[t][:, j, :])
                else:
                    ring = nc.scalar if t % 2 == 0 else nc.sync
                    ring.dma_start(out=xt, in_=x_d[t])
                st[("x", t)] = xt

            def stage_proj_pe(w):
                """Emit QK+V proj matmuls for item w as generators so the
                caller can zipper them with older items' PE work."""
                xt_t, off, nbt = WORK[w]
                ntok = nbt * T
                xt = st[("x", xt_t)]
                s = st.setdefault(w, {})

                def qk_mms():
                    qk_ps = qkps.tile([128, NTOK], F32, tag="qk")
                    s["qk_ps"] = qk_ps
                    for j in range(KCH):
                        nc.tensor.matmul(
                            qk_ps[:, 0:ntok],
                            w_sb[:, j, 0:128],
                            xt[:, j, off : off + ntok],
                            start=(j == 0),
                            stop=(j == KCH - 1),
                            skip_group_check=True,
                        )
                        yield

                def v_mms():
                    # V in NATURAL [t, b, h] layout
                    v_ps = vps.tile([T, NBT, H], F32, tag="v")
                    s["v_ps"] = v_ps
                    for b in range(nbt):
                        for j in range(KCH):
                            nc.tensor.matmul(
                                v_ps[:, b, :],
                                xt[:, j, off + T * b : off + T * b + T],
                                w_sb[:, j, 128:192],
                                start=(j == 0),
                                stop=(j == KCH - 1),
                                skip_group_check=True,
                            )
                            yield

                return qk_mms(), v_mms()

            def stage_evac_qk(w):
                """DVE/ACT evacuation of item w's QK projection PSUM, and
                the Pool pre-write of the causal bias into the scores
                PSUM (runs during the proj phase, off the critical
                chain)."""
                _, _, nbt = WORK[w]
                ntok = nbt * T
                s = st[w]
                s_ps = sps.tile([T, NBT, T], F32, tag="s", name="s_ps")
                s["s_ps"] = s_ps
                q_sb = qpool.tile([64, NTOK], BF16, tag="q")
                nc.vector.tensor_copy(q_sb[:, 0:ntok], s["qk_ps"][0:64, 0:ntok])
                k_sb = kpool.tile([64, NTOK], BF16, tag="k")
                nc.scalar.copy(out=k_sb[:, 0:ntok], in_=s["qk_ps"][64:128, 0:ntok])
                s["q"], s["k"] = q_sb, k_sb

            def stage_evac_vt(w):
                _, _, nbt = WORK[w]
                s = st[w]
                v_sb = vpool.tile([T, NBT, H + 1], BF16, tag="v", name="v_sb")
                nc.vector.tensor_copy(
                    v_sb[:, 0:nbt, 0:H], s["v_ps"][:, 0:nbt, :]
                )
                nc.gpsimd.memset(v_sb[:, 0:nbt, H : H + 1], 1.0)
                s["v"] = v_sb

            def stage_mid_pe(w):
                """Transposes + score matmuls for item w (generator)."""
                _, _, nbt = WORK[w]
                s = st[w]

                def mms():
                    # transposed scores: S^T[s,t] per batch, K=64 contraction
                    s_ps = s["s_ps"]
                    for b in range(nbt):
                        nc.tensor.matmul(
                            s_ps[:, b, :],
                            s["k"][:, T * b : T * b + T],
                            s["q"][:, T * b : T * b + T],
                            start=True,
                            stop=True,
                            skip_group_check=True,
                        )
                        yield

                return mms()

            def stage_mid_post(w):
                """exp+mask (ACT/GPSIMD) and V-natural copy for item w.

                For the LAST item the exp is split into half-batches and
                the causal mask runs as a DVE multiply with the constant
                mask tile: the drain chain is latency-bound, and this
                shortens exp->mask->PV for the first half while keeping
                the (busy) Pool engine out of the critical path."""
                _, _, nbt = WORK[w]
                s = st[w]
                p_sb = ppool.tile([T, NBT, T], BF16, tag="p")
                s["p"] = p_sb
                if w < NW - 1:
                    halves = [(0, nbt)]
                else:
                    halves = [(0, nbt // 2), (nbt // 2, nbt)]
                for b0, b1 in halves:
                    nc.scalar.activation(
                        out=p_sb[:, b0:b1, :],
                        in_=s["s_ps"][:, b0:b1, :],
                        func=mybir.ActivationFunctionType.Exp,
                        scale=0.125,
                    )
                    # causal: keep where t - s >= 0 (layout [s, b, t])
                    nc.gpsimd.affine_select(
                        out=p_sb[:, b0:b1, :],
                        in_=p_sb[:, b0:b1, :],
                        pattern=[[0, b1 - b0], [1, T]],
                        compare_op=mybir.AluOpType.is_ge,
                        fill=0.0,
                        base=0,
                        channel_multiplier=-1,
                    )


            def stage_back_pe(w):
                """PV matmuls for item w (generator)."""
                _, _, nbt = WORK[w]
                s = st[w]

                def mms():
                    o_ps = ops.tile([T, NBT, H + 1], F32, tag="o")
                    s["o_ps"] = o_ps
                    for b in range(nbt):
                        nc.tensor.matmul(
                            o_ps[:, b, :],
                            s["p"][:, b, :],
                            s["v"][:, b, :],
                            start=True,
                            stop=True,
                            skip_group_check=True,
                        )
                        yield

                return mms()

            def stage_back_post(w):
                _, _, nbt = WORK[w]
                s = st[w]
                o_ps = s["o_ps"]
                r_sb = rpool.tile([T, NBT, 1], F32, tag="r")
                o_sb = opool.tile([T, NBT, H], BF16, tag="osb")
                if w < NW - 1:
                    halves = [(0, nbt)]
                else:
                    halves = [(0, nbt // 2), (nbt // 2, nbt)]
                for b0, b1 in halves:
                    nc.vector.reciprocal(
                        out=r_sb[:, b0:b1], in_=o_ps[:, b0:b1, H : H + 1]
                    )
                    nc.vector.tensor_mul(
                        o_sb[:, b0:b1, :],
                        o_ps[:, b0:b1, 0:H],
                        r_sb[:, b0:b1].to_broadcast([T, b1 - b0, H]),
                    )
                    nc.sync.dma_start(
                        out=out_d[w][:, b0:b1, :], in_=o_sb[:, b0:b1, :]
                    )
                del st[w]

            def chain(gen):
                if gen is not None:
                    for _ in gen:
                        pass

            def zipper(a, b):
                """Alternate PE emission between two generators."""
                gens = [g for g in (a, b) if g is not None]
                while gens:
                    alive = []
                    for g in gens:
                        try:
                            next(g)
                            alive.append(g)
                        except StopIteration:
                            pass
                    gens = alive

            loop = (
                tc.For_i(0, reps, 1, hint_engines=tuple(nc.engines))
                if reps > 1
                else contextlib.nullcontext()
            )
            with loop:
                for i in range(NW + 2):
                    if i == 0:
                        dma_in(0, split=True)
                        dma_in(1)
                    if i < NW:
                        # prefetch x-tiles two and three work items ahead
                        dma_in(WORK[min(i + 2, NW - 1)][0])
                        dma_in(WORK[min(i + 3, NW - 1)][0])
                    # PE order: PV (old, dep-safe) first, QK chunks
                    # back-to-back (their LDWEIGHTS hide under each
                    # other's streams), then V proj zippered with the
                    # score matmuls + transposes.
                    # DVE queue order: q evac first (scores need it
                    # mid-iteration), then the back-stage normalize
                    # (frees o_ps for the NEXT iteration's PV), then
                    # the vt evac (feeds the late transposes).
                    if 1 <= i <= NW:
                        stage_evac_qk(i - 1)
                        stage_evac_vt(i - 1)
                    qk = v = sc = None
                    if i < NW:
                        qk, v = stage_proj_pe(i)
                    if 1 <= i <= NW:
                        sc = stage_mid_pe(i - 1)
                    chain(qk)
                    # PV(i-2) AFTER the QK chunks: the exp->mask chain of
                    # item i-2 finishes ~1.1us into this iteration, so a
                    # PV-first order stalls the PE queue head on the mask
                    pv = stage_back_pe(i - 2) if i >= 2 else None
                    chain(pv)
                    if i >= 2:
                        stage_back_post(i - 2)
                    zipper(v, sc)
                    if 1 <= i <= NW:
                        stage_mid_post(i - 1)

    nc.compile()
    return nc


def _prep_inputs(x, Wq, Wk, Wv):
    w = np.concatenate(
        [np.asarray(Wq).T, np.asarray(Wk).T, np.asarray(Wv).T], axis=1
    )  # [C, 3H]
    w = np.ascontiguousarray(
        w.reshape(KCH, 128, 3 * H).transpose(1, 0, 2)
    ).astype(ml_dtypes.bfloat16)  # [128, KCH, 3H]
    # x [B, T, C] -> per-core [NT, 128(c-part), KCH, NTOK], token = b*T + t
    xt = np.asarray(x, dtype=np.float32).reshape(NCORES, NT, NBT, T, KCH, 128)
    xt = np.ascontiguousarray(xt.transpose(0, 1, 5, 4, 2, 3)).astype(
        ml_dtypes.bfloat16
    )
    xt = xt.reshape(NCORES, NT, 128, KCH, NTOK)
    return [{"x": xt[i], "w": w} for i in range(NCORES)]


def _run(in_maps, **kw):
    if "nc" not in _cache:
        _cache["nc"] = _build()
    return run_bass_kernel_spmd(
        _cache["nc"], in_maps, core_ids=list(range(NCORES)), **kw
    )


def kernel(x, Wq, Wk, Wv):
    res = _run(_prep_inputs(x, Wq, Wk, Wv))
    outs = []
    for r in res.results:
        o = r["out"].astype(np.float32)  # [NW, T, NBT, H]
        per_batch = []
        for w, (_, _, nbt) in enumerate(WORK):
            per_batch.append(o[w, :, 0:nbt, :].transpose(1, 0, 2))  # [nbt, T, H]
        outs.append(np.concatenate(per_batch, axis=0))  # [BPC, T, H]
    return np.ascontiguousarray(np.concatenate(outs, axis=0))
